# revision 1
# baseline (speedup 1.0000x reference)
"""Trainium2 Bass kernel for sparse-conv (gather-GEMM) + BatchNorm + ReLU.

Contract: kernel(**inputs) takes the FULL unsharded inputs of
nn_BaseConvBlock (feats [1M,32] f32, W [27,32,32] f32, gamma/beta [32] f32,
nbr_idx [1M,27] i32, nbr_mask [1M,27] bool) and returns the full
[1M,32] f32 output, computed SPMD across 8 NeuronCores.

Sharding: voxel dim split 8 ways (125000/core, padded to 980 tiles of 128).
feats is replicated per core; W/gamma/beta replicated; BN batch stats
all-reduced on device ([128,2] f32 collective).

Per 128-voxel tile on each core:
  idxm = mask ? idx : 1e6 (DVE select; row 1e6 of the padded table is zero,
  and bounds_check skips those descriptors entirely)
  27x [P,1]-offset indirect DMA gathers -> G [128, 864] f32 (pre-zeroed)
  7x PE transpose -> GT; 7x accumulating matmul (W stationary, 4 offsets
  per 128-contraction block) -> PSUM y [32co, 128vox] at partition 32*(t%4)
  copy into SBUF-resident y_sb [128, T/4*128] (never round-trips HBM)
Then per-channel sum/sumsq (DVE reduce + ACT Square accum), AllReduce,
scale/shift, one fused ACT relu(y*scale+shift), PE transpose back, store.
"""
import contextlib
import ctypes
import sys
import types

import numpy as np

import concourse.bass as bass
import concourse.bacc as bacc
import concourse.tile as tile
from concourse import mybir
from concourse.masks import make_identity

P = 128
K = 27
CIN = COUT = 32
NROWS = 1_000_000
EPS = 1e-5
JBLK = 7
GW = K * CIN
NCORES = 8
T_TILES = 980            # 980*128 = 125440 >= 125000 per core
dt = mybir.dt
Alu = mybir.AluOpType
Act = mybir.ActivationFunctionType


def _build(n_tiles, n_total, idx_chunk=24):
    nc = bacc.Bacc("TRN2", num_devices=NCORES)
    feats = nc.declare_dram_parameter(
        "feats", [NROWS + 8, CIN], dt.float32, isOutput=False)
    idx_d = nc.declare_dram_parameter(
        "idx_sh", [P, n_tiles, K], dt.int32, isOutput=False)
    msk_d = nc.declare_dram_parameter(
        "mask_sh", [P, n_tiles, K], dt.int32, isOutput=False)
    w_d = nc.declare_dram_parameter(
        "w_stack", [JBLK, P, COUT], dt.float32, isOutput=False)
    gam_d = nc.declare_dram_parameter(
        "gamma_r", [P, 1], dt.float32, isOutput=False)
    bet_d = nc.declare_dram_parameter(
        "beta_r", [P, 1], dt.float32, isOutput=False)
    fold_d = nc.declare_dram_parameter(
        "fold", [P, COUT], dt.float32, isOutput=False)
    out_d = nc.declare_dram_parameter(
        "out_sh", [P, n_tiles * COUT], dt.float32, isOutput=True)

    nq = (n_tiles + 3) // 4
    ycols = nq * P

    with tile.TileContext(nc) as tc:
        with tc.tile_pool(name="const", bufs=1) as cpool, \
             tc.tile_pool(name="ysb", bufs=1) as ypool, \
             tc.tile_pool(name="ix", bufs=3) as ipool, \
             tc.tile_pool(name="g", bufs=6) as gpool, \
             tc.tile_pool(name="gt", bufs=3) as gtpool, \
             tc.tile_pool(name="gtp", bufs=4, space="PSUM") as gtppool, \
             tc.tile_pool(name="yp", bufs=2, space="PSUM") as yppool, \
             tc.tile_pool(name="op", bufs=2, space="PSUM") as oppool, \
             tc.tile_pool(name="st", bufs=2) as stpool, \
             tc.tile_pool(name="dram", bufs=1, space="DRAM") as dpool:

            ident = cpool.tile([P, P], dt.float32)
            make_identity(nc, ident[:])
            wsb = cpool.tile([P, JBLK * COUT], dt.float32)
            nc.sync.dma_start(
                out=wsb[:].rearrange("p (j c) -> p j c", j=JBLK),
                in_=w_d[:].rearrange("j p c -> p j c"))
            gam = cpool.tile([P, 1], dt.float32)
            nc.sync.dma_start(out=gam[:], in_=gam_d[:])
            bet = cpool.tile([P, 1], dt.float32)
            nc.sync.dma_start(out=bet[:], in_=bet_d[:])

            y_sb = ypool.tile([P, ycols], dt.float32)

            # ---- pass 1: gather + conv ----
            for c0 in range(0, n_tiles, idx_chunk):
                c1 = min(c0 + idx_chunk, n_tiles)
                cw = (c1 - c0) * K
                it = ipool.tile([P, idx_chunk * K], dt.int32, tag="it")
                nc.sync.dma_start(
                    out=it[:, :cw],
                    in_=idx_d[:, c0:c1, :].rearrange("p t k -> p (t k)"))
                mt = ipool.tile([P, idx_chunk * K], dt.int32, tag="mt")
                nc.sync.dma_start(
                    out=mt[:, :cw],
                    in_=msk_d[:, c0:c1, :].rearrange("p t k -> p (t k)"))
                im = ipool.tile([P, idx_chunk * K], dt.int32, tag="im")
                nc.vector.memset(im[:, :cw], NROWS)
                nc.vector.copy_predicated(
                    out=im[:, :cw], mask=mt[:, :cw], data=it[:, :cw])

                for t in range(c0, c1):
                    tl = t - c0
                    g = gpool.tile([P, GW], dt.float32, tag="g")
                    nc.vector.memset(g[:], 0)
                    for k in range(K):
                        nc.gpsimd.indirect_dma_start(
                            out=g[:, k * CIN:(k + 1) * CIN],
                            out_offset=None,
                            in_=feats[:],
                            in_offset=bass.IndirectOffsetOnAxis(
                                ap=im[:, tl * K + k:tl * K + k + 1], axis=0),
                            bounds_check=NROWS - 1,
                            oob_is_err=False,
                        )
                    gt = gtpool.tile([P, JBLK * P], dt.float32, tag="gt")
                    for j in range(JBLK):
                        kw = P if j < JBLK - 1 else GW - P * (JBLK - 1)
                        gp = gtppool.tile([P, P], dt.float32, tag="gp")
                        nc.tensor.transpose(
                            out=gp[:kw, :], in_=g[:, j * P:j * P + kw],
                            identity=ident[:])
                        nc.scalar.copy(
                            out=gt[:kw, j * P:(j + 1) * P], in_=gp[:kw, :])
                    q = t % 4
                    yp = yppool.tile([P, P], dt.float32, tag="yp")
                    for j in range(JBLK):
                        kw = P if j < JBLK - 1 else GW - P * (JBLK - 1)
                        nc.tensor.matmul(
                            out=yp[COUT * q:COUT * (q + 1), :],
                            lhsT=wsb[:kw, j * COUT:(j + 1) * COUT],
                            rhs=gt[:kw, j * P:(j + 1) * P],
                            start=(j == 0), stop=(j == JBLK - 1),
                            tile_position=(0, COUT * q),
                        )
                    nc.vector.tensor_copy(
                        out=y_sb[COUT * q:COUT * (q + 1),
                                 (t // 4) * P:(t // 4 + 1) * P],
                        in_=yp[COUT * q:COUT * (q + 1), :])

            # ---- BN stats ----
            SC = 512
            nchunk = (ycols + SC - 1) // SC
            s1p = cpool.tile([P, nchunk], dt.float32)
            s2p = cpool.tile([P, nchunk], dt.float32)
            scr = cpool.tile([P, SC], dt.float32)
            for ci, c0 in enumerate(range(0, ycols, SC)):
                c1 = min(c0 + SC, ycols)
                nc.vector.tensor_reduce(
                    out=s1p[:, ci:ci + 1], in_=y_sb[:, c0:c1],
                    axis=mybir.AxisListType.X, op=Alu.add)
                nc.scalar.activation(
                    out=scr[:, :c1 - c0], in_=y_sb[:, c0:c1],
                    func=Act.Square, accum_out=s2p[:, ci:ci + 1])
            s12 = cpool.tile([P, 2], dt.float32)
            nc.vector.tensor_reduce(
                out=s12[:, 0:1], in_=s1p[:], axis=mybir.AxisListType.X,
                op=Alu.add)
            nc.vector.tensor_reduce(
                out=s12[:, 1:2], in_=s2p[:], axis=mybir.AxisListType.X,
                op=Alu.add)

            cc_in = dpool.tile([P, 2], dt.float32)
            cc_out = dpool.tile([P, 2], dt.float32)
            nc.sync.dma_start(out=cc_in[:], in_=s12[:])
            nc.gpsimd.collective_compute(
                "AllReduce", Alu.add,
                replica_groups=[list(range(NCORES))],
                ins=[cc_in.opt()], outs=[cc_out.opt()])
            s12r = cpool.tile([P, 2], dt.float32)
            nc.sync.dma_start(out=s12r[:], in_=cc_out[:])

            fold = cpool.tile([P, COUT], dt.float32)
            nc.sync.dma_start(out=fold[:], in_=fold_d[:])
            sfold = oppool.tile([COUT, 2], dt.float32, tag="op")
            nc.tensor.matmul(out=sfold[:], lhsT=fold[:], rhs=s12r[:],
                             start=True, stop=True)
            mv = cpool.tile([COUT, 2], dt.float32)
            nc.vector.tensor_scalar_mul(mv[:], sfold[:], 1.0 / n_total)
            mean2 = cpool.tile([COUT, 1], dt.float32)
            nc.vector.tensor_tensor(
                out=mean2[:], in0=mv[:, 0:1], in1=mv[:, 0:1], op=Alu.mult)
            var = cpool.tile([COUT, 1], dt.float32)
            nc.vector.tensor_tensor(
                out=var[:], in0=mv[:, 1:2], in1=mean2[:], op=Alu.subtract)
            eps_t = cpool.tile([COUT, 1], dt.float32)
            nc.vector.memset(eps_t[:], EPS)
            std = cpool.tile([COUT, 1], dt.float32)
            nc.scalar.activation(out=std[:], in_=var[:], func=Act.Sqrt,
                                 bias=eps_t[:])
            rstd = cpool.tile([COUT, 1], dt.float32)
            nc.vector.reciprocal(out=rstd[:], in_=std[:])
            pk = cpool.tile([COUT, 2], dt.float32)
            nc.vector.tensor_copy(out=pk[:, 0:1], in_=mv[:, 0:1])
            nc.vector.tensor_copy(out=pk[:, 1:2], in_=rstd[:])
            mr_d = dpool.tile([COUT, 2], dt.float32)
            nc.sync.dma_start(out=mr_d[:], in_=pk[:])
            mr = cpool.tile([P, 2], dt.float32)
            for q in range(4):
                nc.sync.dma_start(
                    out=mr[COUT * q:COUT * (q + 1), :], in_=mr_d[:])
            scale = cpool.tile([P, 1], dt.float32)
            nc.vector.tensor_tensor(
                out=scale[:], in0=gam[:], in1=mr[:, 1:2], op=Alu.mult)
            shift = cpool.tile([P, 1], dt.float32)
            nc.vector.tensor_tensor(
                out=shift[:], in0=mr[:, 0:1], in1=scale[:], op=Alu.mult)
            nc.vector.tensor_tensor(
                out=shift[:], in0=bet[:], in1=shift[:], op=Alu.subtract)

            # ---- pass 2: normalize + relu + transpose + store ----
            nc.scalar.activation(
                out=y_sb[:], in_=y_sb[:], func=Act.Relu,
                scale=scale[:], bias=shift[:])
            ochunk = 32
            for c0 in range(0, n_tiles, ochunk):
                c1 = min(c0 + ochunk, n_tiles)
                st = stpool.tile([P, ochunk * COUT], dt.float32, tag="st")
                for t in range(c0, c1):
                    q = t % 4
                    op = oppool.tile([P, COUT], dt.float32, tag="op")
                    nc.tensor.transpose(
                        out=op[:],
                        in_=y_sb[COUT * q:COUT * (q + 1),
                                 (t // 4) * P:(t // 4 + 1) * P],
                        identity=ident[COUT * q:COUT * (q + 1),
                                       COUT * q:COUT * (q + 1)],
                        tile_position=(COUT * q, 0))
                    nc.scalar.copy(
                        out=st[:, (t - c0) * COUT:(t - c0 + 1) * COUT],
                        in_=op[:])
                nc.sync.dma_start(
                    out=out_d[:, c0 * COUT:c1 * COUT],
                    in_=st[:, :(c1 - c0) * COUT])
    return nc


def _install_ntff_hook():
    """The container's antenv lacks axon_hooks; provide it so trace=True
    works (harmless if never used)."""
    if "antenv.axon_hooks" in sys.modules:
        return
    try:
        lib = ctypes.CDLL("/opt/axon/libaxon_pjrt.so")
        lib.axon_start_nrt_profile.argtypes = [
            ctypes.POINTER(ctypes.c_int64), ctypes.c_size_t]
        lib.axon_start_nrt_profile.restype = ctypes.c_int64
        lib.axon_stop_nrt_profile.argtypes = [ctypes.c_char_p]
        lib.axon_stop_nrt_profile.restype = ctypes.c_int64
    except OSError:
        return

    @contextlib.contextmanager
    def _hook(output_dir, device_ids):
        import jax
        jax.devices()
        if device_ids:
            ids = (ctypes.c_int64 * len(device_ids))(*device_ids)
            rc = lib.axon_start_nrt_profile(ids, len(device_ids))
        else:
            rc = lib.axon_start_nrt_profile(None, 0)
        if rc != 0:
            raise RuntimeError(f"axon_start_nrt_profile rc={rc}")
        try:
            yield
        finally:
            n = lib.axon_stop_nrt_profile(str(output_dir).encode())
            if n <= 0:
                print(f"profile: {n} files in {output_dir}", file=sys.stderr)

    mod = types.ModuleType("antenv.axon_hooks")
    mod.get_axon_ntff_profile_hook = lambda: _hook
    mod.set_axon_ntff_profile_hook = lambda h: None
    sys.modules["antenv.axon_hooks"] = mod


_NC_CACHE = {}


def _get_nc():
    if "nc" not in _NC_CACHE:
        _NC_CACHE["nc"] = _build(T_TILES, NROWS)
        _NC_CACHE["nc"].finalize()
    return _NC_CACHE["nc"]


def kernel(feats, W, gamma, beta, nbr_idx, nbr_mask, trace=False):
    feats = np.asarray(feats, np.float32)
    W = np.asarray(W, np.float32)
    gamma = np.asarray(gamma, np.float32)
    beta = np.asarray(beta, np.float32)
    nbr_idx = np.asarray(nbr_idx, np.int32)
    nbr_mask = np.asarray(nbr_mask).astype(np.int32)
    n = feats.shape[0]
    assert n == NROWS and n % NCORES == 0

    # host layout prep (data-independent)
    feats_pad = np.zeros((NROWS + 8, CIN), np.float32)
    feats_pad[:n] = feats
    w_stack = np.zeros((JBLK, P, COUT), np.float32)
    for k in range(K):
        j, m = k // 4, k % 4
        w_stack[j, 32 * m:32 * (m + 1), :] = W[k]
    gamma_r = np.tile(gamma.reshape(COUT, 1), (4, 1))
    beta_r = np.tile(beta.reshape(COUT, 1), (4, 1))
    fold = np.tile(np.eye(COUT, dtype=np.float32), (4, 1))

    per = T_TILES * P
    in_maps = []
    for c in range(NCORES):
        lo, hi = c * (n // NCORES), (c + 1) * (n // NCORES)
        cnt = hi - lo
        idx = np.zeros((per, K), np.int32)
        msk = np.zeros((per, K), np.int32)
        idx[:cnt] = nbr_idx[lo:hi]
        msk[:cnt] = nbr_mask[lo:hi]
        in_maps.append(dict(
            feats=feats_pad,
            idx_sh=np.ascontiguousarray(
                idx.reshape(T_TILES, P, K).transpose(1, 0, 2)),
            mask_sh=np.ascontiguousarray(
                msk.reshape(T_TILES, P, K).transpose(1, 0, 2)),
            w_stack=w_stack, gamma_r=gamma_r, beta_r=beta_r, fold=fold))

    _install_ntff_hook()
    from concourse import bass_utils
    bass_utils.upload_artifacts = lambda tmpdir: tmpdir
    nc = _get_nc()
    res = bass_utils.run_bass_kernel_spmd(
        nc, in_maps, core_ids=list(range(NCORES)), trace=trace)

    chunks = []
    for c in range(NCORES):
        o = res.results[c]["out_sh"].reshape(P, T_TILES, COUT)
        o = o.transpose(1, 0, 2).reshape(per, COUT)
        chunks.append(o[:n // NCORES])
    out = np.concatenate(chunks, axis=0)
    if trace:
        kernel.last_exec_time_ns = res.exec_time_ns
        kernel.last_trace = (res.instructions_and_trace or (None, None))[1]
    return out

